# revision 3
# baseline (speedup 1.0000x reference)
"""Macro-F1 kernel for Trainium2, 8 NeuronCores — stats-only design.

F1 needs only diag(cm), per-class pred counts, and support — not the full
confusion matrix.  Per core (data-parallel over rows), tiles of 16*128 rows
([128p, 16k, 128c], row = base + p*16 + k):
  - DVE:  rowmax via tensor_reduce (X axis), written straight into a
          persistent rmax buffer (shipped to host at the end)
  - ACT:  anti[n,c] = sign(rmax_n - x[n,c]) in {0,1} for M_ACT chunks
  - DVE:  anti for the rest via one broadcast is_lt
  - PE :  colsum += ones[128,1]^T @ anti  (bf16, fp32 PSUM [1,512];
          4 chunks per matmul, 4 partial class histograms)
Host: pred_count = N - colsum;  correct_n = (y_pred[n, t_n] == rmax_n)
(exact fp32 equality — rmax is a copy of some x element);  diag via
bincount(t, correct);  tail rows (72/core) done on host;  macro-F1 epilogue.
"""

import sys
import time

if "/opt/trn_rl_repo" not in sys.path:
    sys.path.insert(0, "/opt/trn_rl_repo")

import numpy as np

import concourse.bacc as bacc
import concourse.mybir as mybir
import concourse.tile as tile
from concourse import bass_utils

C = 128
N = 1_000_000
NCORES = 8
R = N // NCORES          # 125000 rows per core
TK = 16                  # chunks (of 128 rows) per big tile
TR = 128 * TK            # 2048 rows per big tile
NT = R // TR             # 61 big tiles
RD = NT * TR             # 124928 device rows per core (72 tail rows on host)
M_ACT = 10               # anti chunks per tile computed on ACT (rest on DVE)
EPS = 1e-12

_CACHE = {}


def _build():
    f32 = mybir.dt.float32
    bf16 = mybir.dt.bfloat16
    Alu = mybir.AluOpType
    Act = mybir.ActivationFunctionType

    nc = bacc.Bacc("TRN2", target_bir_lowering=False, debug=False,
                   num_devices=NCORES)
    yp = nc.dram_tensor("yp", [RD, C], f32, kind="ExternalInput")
    rmx = nc.dram_tensor("rmx", [128, NT * TK], f32, kind="ExternalOutput")
    cs = nc.dram_tensor("cs", [1, 4 * C], f32, kind="ExternalOutput")

    with tile.TileContext(nc) as tc:
        with (
            tc.tile_pool(name="const", bufs=1) as cpool,
            tc.tile_pool(name="xin", bufs=8) as xpool,
            tc.tile_pool(name="anti", bufs=4) as apool,
            tc.tile_pool(name="small", bufs=4) as spool,
            tc.tile_pool(name="psum", bufs=1, space="PSUM") as psum,
        ):
            ones_bf = cpool.tile([128, 1], bf16)
            nc.vector.memset(ones_bf[:], 1.0)
            rmax_all = cpool.tile([128, NT * TK], f32)
            acc = psum.tile([1, 4 * C], f32)

            for i in range(NT):
                x = xpool.tile([128, TK, C], f32, tag="x")
                nc.sync.dma_start(
                    x[:],
                    yp.ap()[i * TR : (i + 1) * TR, :].rearrange(
                        "(p k) c -> p k c", k=TK
                    ),
                )
                col0 = i * TK
                rmax = rmax_all[:, col0 : col0 + TK]
                nc.vector.tensor_reduce(
                    rmax, x[:], axis=mybir.AxisListType.X, op=Alu.max
                )
                anti = apool.tile([128, TK, C], bf16, tag="anti")
                for k in range(M_ACT):
                    nc.scalar.activation(
                        anti[:, k, :], x[:, k, :], Act.Sign,
                        bias=rmax_all[:, col0 + k : col0 + k + 1], scale=-1.0,
                    )
                nc.vector.tensor_tensor(
                    anti[:, M_ACT:TK, :], x[:, M_ACT:TK, :],
                    rmax_all[:, col0 + M_ACT : col0 + TK, None].broadcast_to(
                        [128, TK - M_ACT, C]
                    ),
                    op=Alu.is_lt,
                )
                for g in range(4):
                    nc.tensor.matmul(
                        acc[:],
                        ones_bf[:],
                        anti[:, 4 * g : 4 * (g + 1), :].rearrange(
                            "p k c -> p (k c)"
                        ),
                        start=(i == 0 and g == 0),
                        stop=(i == NT - 1 and g == 3),
                    )

            cs_sb = spool.tile([1, 4 * C], f32, tag="cs")
            nc.scalar.copy(cs_sb[:], acc[:])
            nc.sync.dma_start(cs.ap()[:], cs_sb[:])
            nc.sync.dma_start(rmx.ap()[:], rmax_all[:])

    nc.compile()
    return nc


def _get_nc():
    if "nc" not in _CACHE:
        _CACHE["nc"] = _build()
    return _CACHE["nc"]


def _run(y_pred, y_true, trace=False):
    nc = _get_nc()
    y_pred = np.asarray(y_pred)
    if y_pred.dtype != np.float32:
        y_pred = y_pred.astype(np.float32)
    yt = np.asarray(y_true).astype(np.int64)
    in_maps = [
        {"yp": np.ascontiguousarray(y_pred[c * R : c * R + RD])}
        for c in range(NCORES)
    ]
    res = None
    for attempt in range(3):
        try:
            res = bass_utils.run_bass_kernel_spmd(
                nc, in_maps, core_ids=list(range(NCORES)), trace=trace
            )
            break
        except Exception:
            if attempt == 2:
                raise
            time.sleep(2.0)

    support = np.bincount(yt, minlength=C).astype(np.float64)
    pred_count = np.zeros(C, dtype=np.float64)
    diag = np.zeros(C, dtype=np.float64)
    for c in range(NCORES):
        r = res.results[c]
        # device rows: per-class count of anti==1, 4 partial histograms
        colsum = r["cs"].reshape(4, C).sum(axis=0)
        pred_count += RD - colsum
        # rmax[p, i*16+k] corresponds to row i*2048 + p*16 + k
        rmax_rows = (
            r["rmx"].reshape(128, NT, TK).transpose(1, 0, 2).reshape(-1)
        )
        t_dev = yt[c * R : c * R + RD]
        z = np.take_along_axis(
            y_pred[c * R : c * R + RD], t_dev[:, None].astype(np.int64), axis=1
        )[:, 0]
        correct = (z == rmax_rows).astype(np.float64)
        diag += np.bincount(t_dev, weights=correct, minlength=C)
        # tail rows on host
        tail = y_pred[c * R + RD : (c + 1) * R]
        t_tail = yt[c * R + RD : (c + 1) * R]
        pred_tail = np.argmax(tail, axis=1)
        pred_count += np.bincount(pred_tail, minlength=C)
        diag += np.bincount(
            t_tail, weights=(pred_tail == t_tail).astype(np.float64), minlength=C
        )

    precision = diag / (support + EPS)
    recall = diag / (pred_count + EPS)
    f1 = 2.0 * precision * recall / (precision + recall + EPS)
    return np.float32(f1.mean()), res


def kernel(y_pred, y_true):
    out, _ = _run(y_pred, y_true, trace=False)
    return out


# revision 5
# speedup vs baseline: 1.1106x; 1.1106x over previous
"""Macro-F1 kernel for Trainium2, 8 NeuronCores — fp16 stats-only design.

F1 needs only diag(cm), per-class pred counts, and support — not the full
confusion matrix.  y_pred is cast to fp16 on host (rel err vs fp32 reference
~9e-4, far under the 2e-2 gate), halving HBM traffic.  Per core, tiles of
16*128 rows ([128p, 16k, 128c], row = base + p*16 + k):
  - DVE:  rowmax as a max-tree: two packed fp16 tensor_tensor max halvings
          (2x DVE mode) + a small tensor_reduce into a persistent fp32
          rmax buffer (shipped to host at the end)
  - ACT:  anti[n,c] = sign(rmax_n - x[n,c]) in {0,1} for M_ACT chunks
  - DVE:  anti for the rest via per-chunk tensor_scalar is_lt (4x DVE mode)
  - PE :  colsum += ones[128,1]^T @ anti  (bf16, fp32 PSUM [1,512];
          4 chunks per matmul, 4 partial class histograms)
Host: pred_count = N - colsum;  correct_n = (fp32(y_pred16[n, t_n]) ==
rmax_n) — exact, rmax is a copy of some fp16 element;  diag via
bincount(t, correct);  tail rows (72/core) on host;  macro-F1 epilogue.
"""

import sys
import time

if "/opt/trn_rl_repo" not in sys.path:
    sys.path.insert(0, "/opt/trn_rl_repo")

import numpy as np

import concourse.bacc as bacc
import concourse.mybir as mybir
import concourse.tile as tile
from concourse import bass_utils

C = 128
N = 1_000_000
NCORES = 8
R = N // NCORES          # 125000 rows per core
TK = 16                  # chunks (of 128 rows) per big tile
TR = 128 * TK            # 2048 rows per big tile
NT = R // TR             # 61 big tiles
RD = NT * TR             # 124928 device rows per core (72 tail rows on host)
M_ACT = 7                # anti chunks per tile computed on ACT (rest on DVE)
EPS = 1e-12

_CACHE = {}


def _build():
    f16 = mybir.dt.float16
    f32 = mybir.dt.float32
    bf16 = mybir.dt.bfloat16
    Alu = mybir.AluOpType
    Act = mybir.ActivationFunctionType

    nc = bacc.Bacc("TRN2", target_bir_lowering=False, debug=False,
                   num_devices=NCORES)
    yp = nc.dram_tensor("yp", [RD, C], f16, kind="ExternalInput")
    rmx = nc.dram_tensor("rmx", [128, NT * TK], f32, kind="ExternalOutput")
    cs = nc.dram_tensor("cs", [1, 4 * C], f32, kind="ExternalOutput")

    with tile.TileContext(nc) as tc:
        with (
            tc.tile_pool(name="const", bufs=1) as cpool,
            tc.tile_pool(name="xin", bufs=10) as xpool,
            tc.tile_pool(name="anti", bufs=4) as apool,
            tc.tile_pool(name="half", bufs=4) as hpool,
            tc.tile_pool(name="small", bufs=4) as spool,
            tc.tile_pool(name="psum", bufs=1, space="PSUM") as psum,
        ):
            ones_bf = cpool.tile([128, 1], bf16)
            nc.vector.memset(ones_bf[:], 1.0)
            rmax_all = cpool.tile([128, NT * TK], f32)
            acc = psum.tile([1, 4 * C], f32)

            for i in range(NT):
                x = xpool.tile([128, TK, C], f16, tag="x")
                nc.sync.dma_start(
                    x[:],
                    yp.ap()[i * TR : (i + 1) * TR, :].rearrange(
                        "(p k) c -> p k c", k=TK
                    ),
                )
                col0 = i * TK
                h1 = hpool.tile([128, TK, 64], f16, tag="h1")
                nc.vector.tensor_tensor(
                    h1[:], x[:, :, 0:64], x[:, :, 64:128], op=Alu.max
                )
                h2 = hpool.tile([128, TK, 32], f16, tag="h2")
                nc.vector.tensor_tensor(
                    h2[:], h1[:, :, 0:32], h1[:, :, 32:64], op=Alu.max
                )
                rmax = rmax_all[:, col0 : col0 + TK]
                nc.vector.tensor_reduce(
                    rmax, h2[:], axis=mybir.AxisListType.X, op=Alu.max
                )
                anti = apool.tile([128, TK, C], bf16, tag="anti")
                for k in range(M_ACT):
                    nc.scalar.activation(
                        anti[:, k, :], x[:, k, :], Act.Sign,
                        bias=rmax_all[:, col0 + k : col0 + k + 1], scale=-1.0,
                    )
                for k in range(M_ACT, TK):
                    nc.vector.tensor_scalar(
                        anti[:, k, :], x[:, k, :],
                        rmax_all[:, col0 + k : col0 + k + 1], None,
                        op0=Alu.is_lt,
                    )
                for g in range(4):
                    nc.tensor.matmul(
                        acc[:],
                        ones_bf[:],
                        anti[:, 4 * g : 4 * (g + 1), :].rearrange(
                            "p k c -> p (k c)"
                        ),
                        start=(i == 0 and g == 0),
                        stop=(i == NT - 1 and g == 3),
                    )

            cs_sb = spool.tile([1, 4 * C], f32, tag="cs")
            nc.scalar.copy(cs_sb[:], acc[:])
            nc.sync.dma_start(cs.ap()[:], cs_sb[:])
            nc.sync.dma_start(rmx.ap()[:], rmax_all[:])

    nc.compile()
    return nc


def _get_nc():
    if "nc" not in _CACHE:
        _CACHE["nc"] = _build()
    return _CACHE["nc"]


def _run(y_pred, y_true, trace=False):
    nc = _get_nc()
    y_pred = np.asarray(y_pred)
    if y_pred.dtype != np.float32:
        y_pred = y_pred.astype(np.float32)
    yp16 = y_pred.astype(np.float16)
    yt = np.asarray(y_true).astype(np.int64)
    in_maps = [
        {"yp": np.ascontiguousarray(yp16[c * R : c * R + RD])}
        for c in range(NCORES)
    ]
    res = None
    for attempt in range(3):
        try:
            res = bass_utils.run_bass_kernel_spmd(
                nc, in_maps, core_ids=list(range(NCORES)), trace=trace
            )
            break
        except Exception:
            if attempt == 2:
                raise
            time.sleep(2.0)

    support = np.bincount(yt, minlength=C).astype(np.float64)
    pred_count = np.zeros(C, dtype=np.float64)
    diag = np.zeros(C, dtype=np.float64)
    for c in range(NCORES):
        r = res.results[c]
        # device rows: per-class count of anti==1, 4 partial histograms
        colsum = r["cs"].reshape(4, C).sum(axis=0)
        pred_count += RD - colsum
        # rmax[p, i*16+k] corresponds to row i*2048 + p*16 + k
        rmax_rows = (
            r["rmx"].reshape(128, NT, TK).transpose(1, 0, 2).reshape(-1)
        )
        t_dev = yt[c * R : c * R + RD]
        z = np.take_along_axis(
            yp16[c * R : c * R + RD], t_dev[:, None], axis=1
        )[:, 0].astype(np.float32)
        correct = (z == rmax_rows).astype(np.float64)
        diag += np.bincount(t_dev, weights=correct, minlength=C)
        # tail rows on host (fp16 to match device semantics)
        tail = yp16[c * R + RD : (c + 1) * R]
        t_tail = yt[c * R + RD : (c + 1) * R]
        pred_tail = np.argmax(tail, axis=1)
        pred_count += np.bincount(pred_tail, minlength=C)
        diag += np.bincount(
            t_tail, weights=(pred_tail == t_tail).astype(np.float64), minlength=C
        )

    precision = diag / (support + EPS)
    recall = diag / (pred_count + EPS)
    f1 = 2.0 * precision * recall / (precision + recall + EPS)
    return np.float32(f1.mean()), res


def kernel(y_pred, y_true):
    out, _ = _run(y_pred, y_true, trace=False)
    return out


# revision 6
# speedup vs baseline: 1.2053x; 1.0852x over previous
"""Macro-F1 kernel for Trainium2, 8 NeuronCores — fp16 stats-only design.

F1 needs only diag(cm), per-class pred counts, and support — not the full
confusion matrix.  y_pred is cast to fp16 on host (rel err vs fp32 reference
~9e-4, far under the 2e-2 gate), halving HBM traffic.  Per core, tiles of
TK*128 rows ([128p, TKk, 128c], row = base + p*TK + k):
  - DVE:  rowmax as a max-tree: two packed fp16 tensor_tensor max halvings
          (2x DVE mode) + a small tensor_reduce into a persistent fp32
          rmax buffer (shipped to host at the end)
  - ACT:  anti[n,c] = sign(rmax_n - x[n,c]) in {0,1} for ~half the chunks
  - DVE:  anti for the rest via per-chunk tensor_scalar is_lt
  - PE :  colsum += ones[128,1]^T @ anti  (bf16, fp32 PSUM [1,512];
          4 chunks per matmul, 4 partial class histograms)
Host: pred_count = N - colsum;  correct_n = (fp32(y_pred16[n, t_n]) ==
rmax_n) — exact, rmax is a copy of some fp16 element;  diag via
bincount(t, correct);  tail rows (72/core) on host;  macro-F1 epilogue.
"""

import sys
import time

if "/opt/trn_rl_repo" not in sys.path:
    sys.path.insert(0, "/opt/trn_rl_repo")

import numpy as np

import concourse.bacc as bacc
import concourse.mybir as mybir
import concourse.tile as tile
from concourse import bass_utils

C = 128
N = 1_000_000
NCORES = 8
R = N // NCORES          # 125000 rows per core
TK = 32                  # chunks (of 128 rows) per big tile
TR = 128 * TK            # 4096 rows per big tile
NT = R // TR             # 30 big tiles
MIDK = (R - NT * TR) // 128  # 16 chunks in the mid tile
RD = NT * TR + MIDK * 128    # 124928 device rows per core (72 on host)
EPS = 1e-12

# chunk columns (i*TK+k) per tile, in emission order; ACT takes the first
# M_ACT of each tile's chunks, DVE tensor_scalar the rest
M_ACT_BIG = 16
M_ACT_MID = 8

_CACHE = {}


def _build():
    f16 = mybir.dt.float16
    f32 = mybir.dt.float32
    bf16 = mybir.dt.bfloat16
    Alu = mybir.AluOpType
    Act = mybir.ActivationFunctionType

    nc = bacc.Bacc("TRN2", target_bir_lowering=False, debug=False,
                   num_devices=NCORES)
    yp = nc.dram_tensor("yp", [RD, C], f16, kind="ExternalInput")
    NCOL = NT * TK + MIDK
    rmx = nc.dram_tensor("rmx", [128, NCOL], f32, kind="ExternalOutput")
    cs = nc.dram_tensor("cs", [1, 4 * C], f32, kind="ExternalOutput")

    with tile.TileContext(nc) as tc:
        with (
            tc.tile_pool(name="const", bufs=1) as cpool,
            tc.tile_pool(name="xin", bufs=6) as xpool,
            tc.tile_pool(name="anti", bufs=3) as apool,
            tc.tile_pool(name="half", bufs=3) as hpool,
            tc.tile_pool(name="small", bufs=4) as spool,
            tc.tile_pool(name="psum", bufs=1, space="PSUM") as psum,
        ):
            ones_bf = cpool.tile([128, 1], bf16)
            nc.vector.memset(ones_bf[:], 1.0)
            rmax_all = cpool.tile([128, NCOL], f32)
            acc = psum.tile([1, 4 * C], f32)
            state = {"started": False}

            def emit_tile(base, col0, tk, m_act, last=False):
                x = xpool.tile([128, tk, C], f16, tag="x")
                nc.sync.dma_start(
                    x[:],
                    yp.ap()[base : base + 128 * tk, :].rearrange(
                        "(p k) c -> p k c", k=tk
                    ),
                )
                h1 = hpool.tile([128, tk, 64], f16, tag="h1")
                nc.vector.tensor_tensor(
                    h1[:], x[:, :, 0:64], x[:, :, 64:128], op=Alu.max
                )
                h2 = hpool.tile([128, tk, 32], f16, tag="h2")
                nc.vector.tensor_tensor(
                    h2[:], h1[:, :, 0:32], h1[:, :, 32:64], op=Alu.max
                )
                rmax = rmax_all[:, col0 : col0 + tk]
                nc.vector.tensor_reduce(
                    rmax, h2[:], axis=mybir.AxisListType.X, op=Alu.max
                )
                anti = apool.tile([128, tk, C], bf16, tag="anti")
                for k in range(m_act):
                    nc.scalar.activation(
                        anti[:, k, :], x[:, k, :], Act.Sign,
                        bias=rmax_all[:, col0 + k : col0 + k + 1], scale=-1.0,
                    )
                for k in range(m_act, tk):
                    nc.vector.tensor_scalar(
                        anti[:, k, :], x[:, k, :],
                        rmax_all[:, col0 + k : col0 + k + 1], None,
                        op0=Alu.is_lt,
                    )
                ngroups = tk // 4
                for g in range(ngroups):
                    nc.tensor.matmul(
                        acc[:],
                        ones_bf[:],
                        anti[:, 4 * g : 4 * (g + 1), :].rearrange(
                            "p k c -> p (k c)"
                        ),
                        start=not state["started"],
                        stop=(last and g == ngroups - 1),
                    )
                    state["started"] = True

            for i in range(NT):
                emit_tile(i * TR, i * TK, TK, M_ACT_BIG)
            emit_tile(NT * TR, NT * TK, MIDK, M_ACT_MID, last=True)

            cs_sb = spool.tile([1, 4 * C], f32, tag="cs")
            nc.scalar.copy(cs_sb[:], acc[:])
            nc.sync.dma_start(cs.ap()[:], cs_sb[:])
            nc.sync.dma_start(rmx.ap()[:], rmax_all[:])

    nc.compile()
    return nc


def _get_nc():
    if "nc" not in _CACHE:
        _CACHE["nc"] = _build()
    return _CACHE["nc"]


def _run(y_pred, y_true, trace=False):
    nc = _get_nc()
    y_pred = np.asarray(y_pred)
    if y_pred.dtype != np.float32:
        y_pred = y_pred.astype(np.float32)
    yp16 = y_pred.astype(np.float16)
    yt = np.asarray(y_true).astype(np.int64)
    in_maps = [
        {"yp": np.ascontiguousarray(yp16[c * R : c * R + RD])}
        for c in range(NCORES)
    ]
    res = None
    for attempt in range(3):
        try:
            res = bass_utils.run_bass_kernel_spmd(
                nc, in_maps, core_ids=list(range(NCORES)), trace=trace
            )
            break
        except Exception:
            if attempt == 2:
                raise
            time.sleep(2.0)

    support = np.bincount(yt, minlength=C).astype(np.float64)
    pred_count = np.zeros(C, dtype=np.float64)
    diag = np.zeros(C, dtype=np.float64)
    for c in range(NCORES):
        r = res.results[c]
        # device rows: per-class count of anti==1, 4 partial histograms
        colsum = r["cs"].reshape(4, C).sum(axis=0)
        pred_count += RD - colsum
        # rmax[p, col0+k] corresponds to row 128*col0 + p*tk + k of its tile;
        # big tiles have tk=32, the mid tile tk=16
        rm = r["rmx"]
        parts = []
        big = rm[:, : NT * TK].reshape(128, NT, TK).transpose(1, 0, 2)
        parts.append(big.reshape(-1))
        mid = rm[:, NT * TK :].reshape(128, MIDK).reshape(128, 1, MIDK)
        parts.append(mid.transpose(1, 0, 2).reshape(-1))
        rmax_rows = np.concatenate(parts)
        t_dev = yt[c * R : c * R + RD]
        z = np.take_along_axis(
            yp16[c * R : c * R + RD], t_dev[:, None], axis=1
        )[:, 0].astype(np.float32)
        correct = (z == rmax_rows).astype(np.float64)
        diag += np.bincount(t_dev, weights=correct, minlength=C)
        # tail rows on host (fp16 to match device semantics)
        tail = yp16[c * R + RD : (c + 1) * R]
        t_tail = yt[c * R + RD : (c + 1) * R]
        pred_tail = np.argmax(tail, axis=1)
        pred_count += np.bincount(pred_tail, minlength=C)
        diag += np.bincount(
            t_tail, weights=(pred_tail == t_tail).astype(np.float64), minlength=C
        )

    precision = diag / (support + EPS)
    recall = diag / (pred_count + EPS)
    f1 = 2.0 * precision * recall / (precision + recall + EPS)
    return np.float32(f1.mean()), res


def kernel(y_pred, y_true):
    out, _ = _run(y_pred, y_true, trace=False)
    return out
